# revision 9
# baseline (speedup 1.0000x reference)
"""Multi-head attention forward on 8 Trainium2 NeuronCores (Bass/Tile).

Problem: B=2, N=2048, D=1024, H=16 heads of dh=64, fp32 in/out.

Sharding: tensor-parallel over heads - core c owns heads {2c, 2c+1} (one
128-row feature slice) for projections + attention over all 4096 tokens;
per-1024-token AllToAlls re-shard by token so each core runs the output
projection (full Wo) for its 512 tokens.

Speed recipe vs the fp32r version (316us):
 - fp16 operands everywhere with fp32 PSUM accumulation. fp32r matmuls
   measure ~432ns per 512-free matmul plus 287ns non-FWL LDWEIGHTS; fp16
   streams at full PE rate with fast-weight-load. End-to-end rounding error
   ~1e-3 against a 2e-2 budget.
 - scores: K=64 per head, so the two heads run as concurrent PE row-tiles
   (tile_position (0,0)/(64,0) via base partitions) instead of zero-padding
   the contraction to 128.
 - exp split: ScalarE does true exp for head0 + the tail of head1; VectorE
   computes the leading SCH_COLS of head1 with a Schraudolph bit-trick
   straight into fp16 bit patterns (i16 = round(1477.32*s + 15316), +-3%
   rel err, washes out under softmax normalization).
 - softmax denominators ride as a ones-column in the attn@v lhsT; head1
   uses [ones|v1] so its shifted SBUF copy lands dims on partitions 64..127
   with the denominator row just below them.
 - normalize multiplies run on GpSimd (SBUF-only engine, otherwise idle).
 - 4 AllToAlls of 256KB fp16 (one per 1024 tokens) pipelined behind the
   window loop; only the last is exposed in the tail.
 - single-DMA bulk loads: x / weights are host-permuted to [128, KC, *] so
   each chunk is one dma_start (DMA issue costs ~0.6us queue time each).
"""
from contextlib import ExitStack

import numpy as np

import concourse.bass as bass
import concourse.tile as tile
from concourse import bacc, mybir
from concourse.bass_utils import run_bass_kernel_spmd
from concourse.masks import make_identity

F32 = mybir.dt.float32
F16 = mybir.dt.float16
I16 = mybir.dt.int16

B, N, D, H, DH = 2, 2048, 1024, 16, 64
W = 8                    # cores
TOK = B * N              # 4096 flattened tokens
KC = D // 128            # contraction chunks for projections (8)
NW = TOK // 512          # 512-token n-windows (8)
MCB = N // 128           # m-chunks per batch (16)
NSHIP = 4                # one AllToAll per 1024 tokens

# Schraudolph fast-exp constants (fp16 bit space): i16 = round(A*s + B)
SCH_A = 1024 * 1.4426950408889634
SCH_B = 1024.0 * 15 - 44.0
SCH_COLS = 384           # leading cols of each e1 [128,512] tile on VectorE

_CACHE = {}


def build_bass():
    nc = bacc.Bacc("TRN2", target_bir_lowering=False)

    xT_d = nc.declare_dram_parameter("xT", [128, KC, TOK], F16, isOutput=False)
    wq_d = nc.declare_dram_parameter("wq", [128, KC, 128], F16, isOutput=False)
    wk_d = nc.declare_dram_parameter("wk", [128, KC, 128], F16, isOutput=False)
    wv_d = nc.declare_dram_parameter("wv", [128, KC, 128], F16, isOutput=False)
    wo_d = nc.declare_dram_parameter("wo", [128, KC, D], F16, isOutput=False)
    bqkv_d = nc.declare_dram_parameter("bqkv", [128, 3], F32, isOutput=False)
    out_d = nc.declare_dram_parameter("out", [512, D], F32, isOutput=True)

    a2a_in = [nc.dram_tensor(f"a2a_in{s}", [W, 128, 128], F16)
              for s in range(NSHIP)]
    a2a_out = [nc.dram_tensor(f"a2a_out{s}", [W, 128, 128], F16)
               for s in range(NSHIP)]

    with tile.TileContext(nc) as tc, ExitStack() as ctx:
        sb1 = ctx.enter_context(tc.tile_pool(name="sb1", bufs=1))
        sbe = ctx.enter_context(tc.tile_pool(name="sbe", bufs=2))
        sbx = ctx.enter_context(tc.tile_pool(name="sbx", bufs=4))
        ps_aux = ctx.enter_context(tc.tile_pool(name="ps_aux", bufs=1, space="PSUM"))
        ps_sc = ctx.enter_context(tc.tile_pool(name="ps_sc", bufs=2, space="PSUM"))
        ps_ha = ctx.enter_context(tc.tile_pool(name="ps_ha", bufs=1, space="PSUM"))

        # ---------- constants ----------
        ident = sb1.tile([128, 128], F16, tag="ident")
        make_identity(nc, ident[:])

        # selectors that broadcast softmax denominators across partitions
        sel0 = sb1.tile([65, 128], F16, tag="sel0")
        nc.vector.memset(sel0[:], 0.0)
        nc.vector.memset(sel0[64:65, 0:64], 1.0)
        sel1 = sb1.tile([128, 128], F16, tag="sel1")
        nc.vector.memset(sel1[:], 0.0)
        nc.vector.memset(sel1[32:33, 64:128], 1.0)

        bias = sb1.tile([128, 3], F32, tag="bias")
        nc.sync.dma_start(bias[:], bqkv_d[:])

        # ---------- persistent activations ----------
        qT = sb1.tile([128, TOK], F16, tag="qT")
        kT = sb1.tile([128, TOK], F16, tag="kT")     # rows 0-63 h0, 64-127 h1
        # v_aug[:, gm, 0:65]   = [v0 | ones] (denominator row at out 64)
        # v_aug[:, gm, 65:130] = [v1 | ones] (denominator row at out 64)
        v_aug = sb1.tile([128, 2 * MCB, 130], F16, tag="v_aug")
        nc.vector.memset(v_aug[:, :, 64:65], 1.0)
        nc.vector.memset(v_aug[:, :, 129:130], 1.0)
        heads = sb1.tile([128, TOK], F16, tag="heads")

        wq = sb1.tile([128, KC, 128], F16, tag="wq")
        wk = sb1.tile([128, KC, 128], F16, tag="wk")
        wv = sb1.tile([128, KC, 128], F16, tag="wv")
        wo = sb1.tile([128, KC, D], F16, tag="wo")
        hT = [sb1.tile([128, KC, 128], F16, tag=f"hT{s}", name=f"hT{s}")
              for s in range(NSHIP)]

        def load_x_chunk(t):
            xt = sbx.tile([128, KC, 512], F16, tag="xt", name=f"xt{t}")
            nc.sync.dma_start(xt[:], xT_d[:, :, bass.ts(t, 512)])
            return xt

        def proj_chunk(t, xt):
            """Project 512 tokens (global chunk t): q,k -> qT/kT, v -> v_aug."""
            tsl = bass.ts(t, 512)
            pq = ps_aux.tile([128, 512], F32, tag="pja", name="pq")
            for k in range(KC):
                nc.tensor.matmul(pq[:], wq[:, k, :], xt[:, k, :],
                                 start=(k == 0), stop=(k == KC - 1))
            pk = ps_aux.tile([128, 512], F32, tag="pjb", name="pk")
            for k in range(KC):
                nc.tensor.matmul(pk[:], wk[:, k, :], xt[:, k, :],
                                 start=(k == 0), stop=(k == KC - 1))
            nc.vector.tensor_scalar_add(qT[:, tsl], pq[:], bias[:, 0:1])
            pv = ps_aux.tile([128, 512], F32, tag="pja", name="pv")
            for k in range(KC):
                nc.tensor.matmul(pv[:], wv[:, k, :], xt[:, k, :],
                                 start=(k == 0), stop=(k == KC - 1))
            nc.scalar.add(kT[:, tsl], pk[:], bias[:, 1:2])
            vt = sbe.tile([128, 512], F16, tag="vt", name="vt")
            nc.vector.tensor_scalar_add(vt[:], pv[:], bias[:, 2:3])
            # transpose v 128-token-wise into v_aug rows (PE transpose)
            for i in range(4):
                gm = 4 * t + i
                tag = "pja" if i % 2 == 0 else "pjb"
                tp = ps_aux.tile([128, 128], F16, tag=tag, name="tp")
                nc.tensor.transpose(tp[:], vt[:, bass.ts(i, 128)], ident[:])
                nc.scalar.copy(v_aug[:, gm, 0:64], tp[:, 0:64])
                nc.scalar.copy(v_aug[:, gm, 65:129], tp[:, 64:128])

        # ---------- stage 2 helpers ----------
        def emit_av(pr, last):
            e0, e1f, gm, ha0, ha1, _w = pr
            first = gm % MCB == 0
            nc.tensor.matmul(ha0[:], v_aug[:, gm, 0:65], e0[:],
                             start=first, stop=last)
            nc.tensor.matmul(ha1[:], v_aug[:, gm, 65:130], e1f,
                             start=first, stop=last)

        def emit_window_end(pr):
            """Copy the finished window's attn@v PSUM to SBUF (frees ha)."""
            _, _, _, ha0, ha1, w = pr
            hs0 = sbe.tile([65, 512], F16, tag="hs0", bufs=1)
            hs1 = sbe.tile([128, 512], F16, tag="hs1", bufs=1)
            nc.vector.tensor_copy(hs0[:], ha0[:])
            # v1 dims -> partitions 64..127; denominator row parked at 32
            nc.vector.tensor_copy(hs1[64:128, :], ha1[0:64, :])
            nc.vector.tensor_copy(hs1[32:33, :], ha1[64:65, :])
            return (hs0, hs1, w)

        def emit_normalize_bc(pend):
            hs0, hs1, w = pend
            bc = ps_aux.tile([128, 512], F32, tag="pjb", name="bc")
            nc.tensor.matmul(bc[:], sel0[:], hs0[:], start=True, stop=False)
            nc.tensor.matmul(bc[:], sel1[:], hs1[:], start=False, stop=True)
            bc_s = sbe.tile([128, 512], F32, tag="bc_s", bufs=1)
            nc.vector.reciprocal_approx_fast(bc_s[:], bc[:])
            return bc_s

        def emit_normalize_mul(pend, bc_s, tail=False):
            hs0, hs1, w = pend
            hsl = bass.ts(w, 512)
            eng0 = nc.vector if tail else nc.gpsimd
            eng0.tensor_mul(heads[0:64, hsl], hs0[0:64, :], bc_s[0:64, :])
            nc.gpsimd.tensor_mul(heads[64:128, hsl], hs1[64:128, :],
                                 bc_s[64:128, :])

        def emit_ship(s):
            for j in range(W):
                eng = nc.sync if j % 2 == 0 else nc.gpsimd
                eng.dma_start(a2a_in[s][j],
                              heads[:, bass.ds(1024 * s + 128 * j, 128)])
            nc.gpsimd.collective_compute(
                "AllToAll",
                mybir.AluOpType.bypass,
                ins=[a2a_in[s][:]],
                outs=[a2a_out[s][:]],
                replica_groups=[list(range(W))],
            )

        def emit_outproj(s):
            for j in range(W):
                eng = nc.sync if j % 2 == 0 else nc.gpsimd
                eng.dma_start(hT[s][:, j, :], a2a_out[s][j])
            for dc in range(2):
                op = ps_aux.tile([128, 512], F32, tag=("pja", "pjb")[dc], name="op")
                for j in range(KC):
                    nc.tensor.matmul(op[:], hT[s][:, j, :],
                                     wo[:, j, bass.ts(dc, 512)],
                                     start=(j == 0), stop=(j == KC - 1))
                ot = sbe.tile([128, 512], F32, tag="ot")
                nc.scalar.copy(ot[:], op[:])
                nc.sync.dma_start(out_d[bass.ts(s, 128), bass.ts(dc, 512)], ot[:])

        def emit_exp(sc0, sc1, e0, e1i):
            nc.scalar.activation(e0[:], sc0[:], mybir.ActivationFunctionType.Exp)
            if SCH_COLS:
                nc.vector.tensor_scalar(
                    out=e1i[:, 0:SCH_COLS], in0=sc1[:, 0:SCH_COLS],
                    scalar1=SCH_A, scalar2=SCH_B,
                    op0=mybir.AluOpType.mult, op1=mybir.AluOpType.add)
            if SCH_COLS < 512:
                nc.scalar.activation(
                    e1i[:, SCH_COLS:512].bitcast(F16), sc1[:, SCH_COLS:512],
                    mybir.ActivationFunctionType.Exp)

        # ---------- schedule ----------
        nc.sync.dma_start(wq[:], wq_d[:])
        nc.sync.dma_start(wk[:], wk_d[:])
        nc.sync.dma_start(wv[:], wv_d[:])
        xts = {t: load_x_chunk(t) for t in range(2)}
        for t in range(4):          # batch-0 projections
            if t == 2:
                nc.sync.dma_start(wo[:], wo_d[:])
            if t + 2 < 4:
                xts[t + 2] = load_x_chunk(t + 2)
            proj_chunk(t, xts.pop(t))

        prev = None      # av software pipeline: (e0, e1f, gm, ha0, ha1, w)
        pending = None   # window awaiting normalize: (hs0, hs1, w)
        bc_pend = None   # (pend, bc_s)

        def window(w, pre=None):
            nonlocal prev, pending, bc_pend
            b = w // 4
            nsl = bass.ds(512 * w, 512)
            ha0 = ps_ha.tile([65, 512], F32, tag="ha0", name="ha0")
            ha1 = ps_ha.tile([65, 512], F32, tag="ha1", name="ha1")
            for mc in range(MCB):
                if mc == 0 and pre is not None:
                    pre()
                gm = MCB * b + mc
                msl = bass.ts(gm, 128)
                sc0 = ps_sc.tile([128, 512], F32, tag="sc0", name="sc0")
                sc1 = ps_sc.tile([128, 512], F32, tag="sc1", name="sc1")
                nc.tensor.matmul(sc0[:], kT[0:64, msl], qT[0:64, nsl],
                                 start=True, stop=True)
                nc.tensor.matmul(sc1[:], kT[64:128, msl], qT[64:128, nsl],
                                 start=True, stop=True)
                if prev is not None:
                    last = prev[2] % MCB == MCB - 1
                    emit_av(prev, last)
                    if last:
                        pending = emit_window_end(prev)
                e0 = sbe.tile([128, 512], F16, tag="e0", name="e0")
                e1i = sbe.tile([128, 512], I16, tag="e1", name="e1")
                emit_exp(sc0, sc1, e0, e1i)
                prev = (e0, e1i[:].bitcast(F16), gm, ha0, ha1, w)
                if mc == 3 and pending is not None:
                    bc_pend = (pending, emit_normalize_bc(pending))
                    pending = None
                if mc == 5 and bc_pend is not None:
                    emit_normalize_mul(bc_pend[0], bc_pend[1])
                    pw = bc_pend[0][2]
                    bc_pend = None
                    if pw % 2 == 1:
                        emit_ship(pw // 2)
                if mc == 9 and w % 2 == 1 and w >= 3:
                    emit_outproj((w - 3) // 2)

        for w in range(4):          # batch-0 attention
            window(w, pre=(lambda t=w + 4: xts.__setitem__(t, load_x_chunk(t))))

        # batch-0 epilogue: finish window 3, normalize + ship it while the
        # batch-1 projections run
        emit_av(prev, True)
        pending = emit_window_end(prev)
        prev = None
        bc_s = emit_normalize_bc(pending)
        emit_normalize_mul(pending, bc_s)
        emit_ship(1)
        pending = None
        for t in range(4, 8):       # batch-1 projections
            proj_chunk(t, xts.pop(t))

        for w in range(4, 8):       # batch-1 attention
            window(w)

        # tail: window 7 normalize + final ship + out-projection
        emit_av(prev, True)
        pending = emit_window_end(prev)
        bc_s = emit_normalize_bc(pending)
        emit_normalize_mul(pending, bc_s, tail=True)
        emit_ship(3)
        emit_outproj(3)

    nc.compile()
    return nc


def _to_f16_perm(a):
    """[D, X] fp32 -> [128, KC, X] fp16 with rows regrouped per 128-block."""
    Dd, X = a.shape
    return np.ascontiguousarray(
        a.reshape(KC, 128, X).transpose(1, 0, 2)).astype(np.float16)


def _prep_inputs(x, Wq, bq, Wk, bk, Wv, bv, Wo, bo):
    xT = np.ascontiguousarray(x.reshape(TOK, D).T)
    xTr = _to_f16_perm(xT)
    wor = _to_f16_perm(Wo)
    in_maps = []
    for c in range(W):
        sl = slice(128 * c, 128 * (c + 1))
        bqkv = np.stack([bq[sl] / 8.0, bk[sl], bv[sl]], axis=1).astype(np.float32)
        in_maps.append({
            "xT": xTr,
            "wq": _to_f16_perm(np.ascontiguousarray(Wq[:, sl]) / 8.0),
            "wk": _to_f16_perm(np.ascontiguousarray(Wk[:, sl])),
            "wv": _to_f16_perm(np.ascontiguousarray(Wv[:, sl])),
            "wo": wor,
            "bqkv": np.ascontiguousarray(bqkv),
        })
    return in_maps


def run(x, Wq, bq, Wk, bk, Wv, bv, Wo, bo, **run_kwargs):
    if "nc" not in _CACHE:
        _CACHE["nc"] = build_bass()
    nc = _CACHE["nc"]
    in_maps = _prep_inputs(x, Wq, bq, Wk, bk, Wv, bv, Wo, bo)
    res = run_bass_kernel_spmd(nc, in_maps, list(range(W)), **run_kwargs)
    out = np.empty((TOK, D), np.float32)
    for c in range(W):
        r = res.results[c]["out"]
        for s in range(NSHIP):
            out[1024 * s + 128 * c:1024 * s + 128 * (c + 1)] = \
                r[128 * s:128 * (s + 1)]
    out = out.reshape(B, N, D) + bo.astype(np.float32)
    return out.astype(np.float32), res


def kernel(x, Wq, bq, Wk, bk, Wv, bv, Wo, bo):
    x, Wq, bq, Wk, bk, Wv, bv, Wo, bo = (
        np.asarray(a, dtype=np.float32)
        for a in (x, Wq, bq, Wk, bk, Wv, bv, Wo, bo)
    )
    out, _ = run(x, Wq, bq, Wk, bk, Wv, bv, Wo, bo)
    return out


# revision 14
# speedup vs baseline: 1.1354x; 1.1354x over previous
"""Multi-head attention forward on 8 Trainium2 NeuronCores (Bass/Tile).

Problem: B=2, N=2048, D=1024, H=16 heads of dh=64, fp32 in/out.

Sharding: tensor-parallel over heads - core c owns heads {2c, 2c+1} (one
128-row feature slice) for projections + attention over all 4096 tokens;
per-1024-token AllToAlls re-shard by token so each core runs the output
projection (full Wo) for its 512 tokens.

Speed recipe vs the fp32r version (316us):
 - fp16 operands everywhere with fp32 PSUM accumulation. fp32r matmuls
   measure ~432ns per 512-free matmul plus 287ns non-FWL LDWEIGHTS; fp16
   streams at full PE rate with fast-weight-load. End-to-end rounding error
   ~1e-3 against a 2e-2 budget.
 - scores: K=64 per head, so the two heads run as concurrent PE row-tiles
   (tile_position (0,0)/(64,0) via base partitions) instead of zero-padding
   the contraction to 128.
 - exp split: ScalarE does true exp for head0 + the tail of head1; VectorE
   computes the leading SCH_COLS of head1 with a Schraudolph bit-trick
   straight into fp16 bit patterns (i16 = round(1477.32*s + 15316), +-3%
   rel err, washes out under softmax normalization).
 - softmax denominators ride as a ones-column in the attn@v lhsT; head1
   uses [ones|v1] so its shifted SBUF copy lands dims on partitions 64..127
   with the denominator row just below them.
 - normalize multiplies run on GpSimd (SBUF-only engine, otherwise idle).
 - 4 AllToAlls of 256KB fp16 (one per 1024 tokens) pipelined behind the
   window loop; only the last is exposed in the tail.
 - single-DMA bulk loads: x / weights are host-permuted to [128, KC, *] so
   each chunk is one dma_start (DMA issue costs ~0.6us queue time each).
"""
from contextlib import ExitStack

import numpy as np

import concourse.bass as bass
import concourse.tile as tile
from concourse import bacc, mybir
from concourse.bass_utils import run_bass_kernel_spmd
from concourse.masks import make_identity

F32 = mybir.dt.float32
F16 = mybir.dt.float16
I16 = mybir.dt.int16

B, N, D, H, DH = 2, 2048, 1024, 16, 64
W = 8                    # cores
TOK = B * N              # 4096 flattened tokens
KC = D // 128            # contraction chunks for projections (8)
NW = TOK // 512          # 512-token n-windows (8)
MCB = N // 128           # m-chunks per batch (16)
NSHIP = 4                # one AllToAll per 1024 tokens

# Schraudolph fast-exp constants (fp16 bit space): i16 = round(A*s + B)
SCH_A = 1024 * 1.4426950408889634
SCH_B = 1024.0 * 15 - 44.0
SCH_COLS = 512           # leading cols of each e1 [128,512] tile on VectorE

_CACHE = {}


def build_bass():
    nc = bacc.Bacc("TRN2", target_bir_lowering=False)

    xT_d = nc.declare_dram_parameter("xT", [128, KC, TOK], F16, isOutput=False)
    wq_d = nc.declare_dram_parameter("wq", [128, KC, 128], F16, isOutput=False)
    wk_d = nc.declare_dram_parameter("wk", [128, KC, 128], F16, isOutput=False)
    wv_d = nc.declare_dram_parameter("wv", [128, KC, 128], F16, isOutput=False)
    wo_d = nc.declare_dram_parameter("wo", [128, KC, D], F16, isOutput=False)
    bqkv_d = nc.declare_dram_parameter("bqkv", [128, 3], F32, isOutput=False)
    out_d = nc.declare_dram_parameter("out", [512, D], F32, isOutput=True)

    a2a_in = [nc.dram_tensor(f"a2a_in{s}", [W, 128, 128], F16)
              for s in range(NSHIP)]
    a2a_out = [nc.dram_tensor(f"a2a_out{s}", [W, 128, 128], F16)
               for s in range(NSHIP)]

    with tile.TileContext(nc) as tc, ExitStack() as ctx:
        sb1 = ctx.enter_context(tc.tile_pool(name="sb1", bufs=1))
        sbe = ctx.enter_context(tc.tile_pool(name="sbe", bufs=2))
        sbx = ctx.enter_context(tc.tile_pool(name="sbx", bufs=4))
        ps_sc = ctx.enter_context(tc.tile_pool(name="ps_sc", bufs=3, space="PSUM"))
        ps_ha = ctx.enter_context(tc.tile_pool(name="ps_ha", bufs=1, space="PSUM"))
        ps_aux = ps_sc   # pj/tp/bc/op borrow the score rings ("sc0"/"sc1")

        # ---------- constants ----------
        ident = sb1.tile([128, 128], F16, tag="ident")
        make_identity(nc, ident[:])

        # selectors that broadcast softmax denominators across partitions
        sel0 = sb1.tile([65, 128], F16, tag="sel0")
        nc.vector.memset(sel0[:], 0.0)
        nc.vector.memset(sel0[64:65, 0:64], 1.0)
        sel1 = sb1.tile([128, 128], F16, tag="sel1")
        nc.vector.memset(sel1[:], 0.0)
        nc.vector.memset(sel1[32:33, 64:128], 1.0)

        bias = sb1.tile([128, 3], F32, tag="bias")
        nc.sync.dma_start(bias[:], bqkv_d[:])

        # ---------- persistent activations ----------
        qT = sb1.tile([128, TOK], F16, tag="qT")
        kT = sb1.tile([128, TOK], F16, tag="kT")     # rows 0-63 h0, 64-127 h1
        # v_aug[:, gm, 0:65]   = [v0 | ones] (denominator row at out 64)
        # v_aug[:, gm, 65:130] = [v1 | ones] (denominator row at out 64)
        v_aug = sb1.tile([128, 2 * MCB, 130], F16, tag="v_aug")
        nc.vector.memset(v_aug[:, :, 64:65], 1.0)
        nc.vector.memset(v_aug[:, :, 129:130], 1.0)
        heads = sb1.tile([128, TOK], F16, tag="heads")

        wq = sb1.tile([128, KC, 128], F16, tag="wq")
        wk = sb1.tile([128, KC, 128], F16, tag="wk")
        wv = sb1.tile([128, KC, 128], F16, tag="wv")
        wo = sb1.tile([128, KC, D], F16, tag="wo")
        hT = [sb1.tile([128, KC, 128], F16, tag=f"hT{s}", name=f"hT{s}")
              for s in range(NSHIP)]

        def load_x_chunk(t):
            xt = sbx.tile([128, KC, 512], F16, tag="xt", name=f"xt{t}")
            nc.sync.dma_start(xt[:], xT_d[:, :, bass.ts(t, 512)])
            return xt

        def proj_chunk(t, xt):
            """Project 512 tokens (global chunk t): q,k -> qT/kT, v -> v_aug."""
            tsl = bass.ts(t, 512)
            pq = ps_aux.tile([128, 512], F32, tag="sc0", name="pq")
            for k in range(KC):
                nc.tensor.matmul(pq[:], wq[:, k, :], xt[:, k, :],
                                 start=(k == 0), stop=(k == KC - 1))
            pk = ps_aux.tile([128, 512], F32, tag="sc1", name="pk")
            for k in range(KC):
                nc.tensor.matmul(pk[:], wk[:, k, :], xt[:, k, :],
                                 start=(k == 0), stop=(k == KC - 1))
            nc.vector.tensor_scalar_add(qT[:, tsl], pq[:], bias[:, 0:1])
            pv = ps_aux.tile([128, 512], F32, tag="sc0", name="pv")
            for k in range(KC):
                nc.tensor.matmul(pv[:], wv[:, k, :], xt[:, k, :],
                                 start=(k == 0), stop=(k == KC - 1))
            nc.scalar.add(kT[:, tsl], pk[:], bias[:, 1:2])
            vt = sbe.tile([128, 512], F16, tag="vt", name="vt")
            nc.vector.tensor_scalar_add(vt[:], pv[:], bias[:, 2:3])
            # transpose v 128-token-wise into v_aug rows (PE transpose)
            for i in range(4):
                gm = 4 * t + i
                tag = "sc0" if i % 2 == 0 else "sc1"
                tp = ps_aux.tile([128, 128], F16, tag=tag, name="tp")
                nc.tensor.transpose(tp[:], vt[:, bass.ts(i, 128)], ident[:])
                nc.scalar.copy(v_aug[:, gm, 0:64], tp[:, 0:64])
                nc.scalar.copy(v_aug[:, gm, 65:129], tp[:, 64:128])

        # ---------- stage 2 helpers ----------
        def emit_av(pr, last):
            e0, e1f, gm, ha0, ha1, _w = pr
            first = gm % MCB == 0
            # h1 first: its e comes from the (earlier-finishing) VectorE path
            nc.tensor.matmul(ha1[:], v_aug[:, gm, 65:130], e1f,
                             start=first, stop=last)
            nc.tensor.matmul(ha0[:], v_aug[:, gm, 0:65], e0[:],
                             start=first, stop=last)

        def emit_window_end(pr):
            """Copy the finished window's attn@v PSUM to SBUF (frees ha)."""
            _, _, _, ha0, ha1, w = pr
            hs0 = sbe.tile([65, 512], F16, tag="hs0", bufs=1)
            hs1 = sbe.tile([128, 512], F16, tag="hs1", bufs=1)
            nc.vector.tensor_copy(hs0[:], ha0[:])
            # v1 dims -> partitions 64..127; denominator row parked at 32
            nc.vector.tensor_copy(hs1[64:128, :], ha1[0:64, :])
            nc.vector.tensor_copy(hs1[32:33, :], ha1[64:65, :])
            return (hs0, hs1, w)

        def emit_normalize_bc(pend):
            hs0, hs1, w = pend
            bc = ps_aux.tile([128, 512], F32, tag="sc1", name="bc")
            nc.tensor.matmul(bc[:], sel0[:], hs0[:], start=True, stop=False)
            nc.tensor.matmul(bc[:], sel1[:], hs1[:], start=False, stop=True)
            bc_s = sbe.tile([128, 512], F32, tag="bc_s", bufs=1)
            nc.vector.reciprocal_approx_fast(bc_s[:], bc[:])
            return bc_s

        def emit_normalize_mul(pend, bc_s, tail=False):
            hs0, hs1, w = pend
            hsl = bass.ts(w, 512)
            eng0 = nc.vector if tail else nc.gpsimd
            eng0.tensor_mul(heads[0:64, hsl], hs0[0:64, :], bc_s[0:64, :])
            nc.gpsimd.tensor_mul(heads[64:128, hsl], hs1[64:128, :],
                                 bc_s[64:128, :])

        def emit_ship(s):
            for j in range(W):
                eng = nc.sync if j % 2 == 0 else nc.gpsimd
                eng.dma_start(a2a_in[s][j],
                              heads[:, bass.ds(1024 * s + 128 * j, 128)])
            nc.gpsimd.collective_compute(
                "AllToAll",
                mybir.AluOpType.bypass,
                ins=[a2a_in[s][:]],
                outs=[a2a_out[s][:]],
                replica_groups=[list(range(W))],
            )

        def emit_outproj(s):
            for j in range(W):
                eng = nc.sync if j % 2 == 0 else nc.gpsimd
                eng.dma_start(hT[s][:, j, :], a2a_out[s][j])
            for dc in range(2):
                op = ps_aux.tile([128, 512], F32, tag=("sc0", "sc1")[dc], name="op")
                for j in range(KC):
                    nc.tensor.matmul(op[:], hT[s][:, j, :],
                                     wo[:, j, bass.ts(dc, 512)],
                                     start=(j == 0), stop=(j == KC - 1))
                ot = sbe.tile([128, 512], F32, tag="ot")
                nc.scalar.copy(ot[:], op[:])
                nc.sync.dma_start(out_d[bass.ts(s, 128), bass.ts(dc, 512)], ot[:])

        def emit_exp(sc0, sc1, e0, e1i):
            nc.scalar.activation(e0[:], sc0[:], mybir.ActivationFunctionType.Exp)
            if SCH_COLS:
                nc.vector.tensor_scalar(
                    out=e1i[:, 0:SCH_COLS], in0=sc1[:, 0:SCH_COLS],
                    scalar1=SCH_A, scalar2=SCH_B,
                    op0=mybir.AluOpType.mult, op1=mybir.AluOpType.add)
            if SCH_COLS < 512:
                nc.scalar.activation(
                    e1i[:, SCH_COLS:512].bitcast(F16), sc1[:, SCH_COLS:512],
                    mybir.ActivationFunctionType.Exp)

        # ---------- schedule ----------
        nc.sync.dma_start(wq[:], wq_d[:])
        nc.sync.dma_start(wk[:], wk_d[:])
        nc.sync.dma_start(wv[:], wv_d[:])
        xts = {t: load_x_chunk(t) for t in range(2)}
        for t in range(4):          # batch-0 projections
            if t == 2:
                nc.sync.dma_start(wo[:], wo_d[:])
            if t + 2 < 4:
                xts[t + 2] = load_x_chunk(t + 2)
            proj_chunk(t, xts.pop(t))

        prev = None      # av software pipeline: (e0, e1f, gm, ha0, ha1, w)
        pending = None   # window awaiting normalize: (hs0, hs1, w)
        bc_pend = None   # (pend, bc_s)

        def window(w, pre=None):
            nonlocal prev, pending, bc_pend
            b = w // 4
            nsl = bass.ds(512 * w, 512)
            ha0 = ps_ha.tile([65, 512], F32, tag="ha0", name="ha0")
            ha1 = ps_ha.tile([65, 512], F32, tag="ha1", name="ha1")
            for mc in range(MCB):
                if mc == 0 and pre is not None:
                    pre()
                gm = MCB * b + mc
                msl = bass.ts(gm, 128)
                # av(prev) first: its deps resolved an iteration ago, so the
                # PE queue flows without stalling; then the adjacent score
                # pair can run as concurrent row-tiles.
                if prev is not None:
                    last = prev[2] % MCB == MCB - 1
                    emit_av(prev, last)
                    if last:
                        pending = emit_window_end(prev)
                sc0 = ps_sc.tile([128, 512], F32, tag="sc0", name="sc0")
                sc1 = ps_sc.tile([128, 512], F32, tag="sc1", name="sc1")
                nc.tensor.matmul(sc0[:], kT[0:64, msl], qT[0:64, nsl],
                                 start=True, stop=True)
                nc.tensor.matmul(sc1[:], kT[64:128, msl], qT[64:128, nsl],
                                 start=True, stop=True)
                e0 = sbe.tile([128, 512], F16, tag="e0", name="e0")
                e1i = sbe.tile([128, 512], I16, tag="e1", name="e1")
                emit_exp(sc0, sc1, e0, e1i)
                prev = (e0, e1i[:].bitcast(F16), gm, ha0, ha1, w)
                if mc == 3 and pending is not None:
                    bc_pend = (pending, emit_normalize_bc(pending))
                    pending = None
                if mc == 5 and bc_pend is not None:
                    emit_normalize_mul(bc_pend[0], bc_pend[1])
                    pw = bc_pend[0][2]
                    bc_pend = None
                    if pw % 2 == 1:
                        emit_ship(pw // 2)
                if mc == 9 and w % 2 == 1 and w >= 3:
                    emit_outproj((w - 3) // 2)

        for w in range(4):          # batch-0 attention
            window(w, pre=(lambda t=w + 4: xts.__setitem__(t, load_x_chunk(t))))

        # batch-0 epilogue: finish window 3, normalize + ship it while the
        # batch-1 projections run
        emit_av(prev, True)
        pending = emit_window_end(prev)
        prev = None
        bc_s = emit_normalize_bc(pending)
        emit_normalize_mul(pending, bc_s)
        emit_ship(1)
        pending = None
        for t in range(4, 8):       # batch-1 projections
            proj_chunk(t, xts.pop(t))

        for w in range(4, 8):       # batch-1 attention
            window(w)

        # tail: window 7 normalize + final ship + out-projection
        emit_av(prev, True)
        pending = emit_window_end(prev)
        bc_s = emit_normalize_bc(pending)
        emit_normalize_mul(pending, bc_s, tail=True)
        emit_ship(3)
        emit_outproj(3)

    nc.compile()
    return nc


def _to_f16_perm(a):
    """[D, X] fp32 -> [128, KC, X] fp16 with rows regrouped per 128-block."""
    Dd, X = a.shape
    return np.ascontiguousarray(
        a.reshape(KC, 128, X).transpose(1, 0, 2)).astype(np.float16)


def _prep_inputs(x, Wq, bq, Wk, bk, Wv, bv, Wo, bo):
    xT = np.ascontiguousarray(x.reshape(TOK, D).T)
    xTr = _to_f16_perm(xT)
    wor = _to_f16_perm(Wo)
    in_maps = []
    for c in range(W):
        sl = slice(128 * c, 128 * (c + 1))
        bqkv = np.stack([bq[sl] / 8.0, bk[sl], bv[sl]], axis=1).astype(np.float32)
        in_maps.append({
            "xT": xTr,
            "wq": _to_f16_perm(np.ascontiguousarray(Wq[:, sl]) / 8.0),
            "wk": _to_f16_perm(np.ascontiguousarray(Wk[:, sl])),
            "wv": _to_f16_perm(np.ascontiguousarray(Wv[:, sl])),
            "wo": wor,
            "bqkv": np.ascontiguousarray(bqkv),
        })
    return in_maps


def run(x, Wq, bq, Wk, bk, Wv, bv, Wo, bo, **run_kwargs):
    if "nc" not in _CACHE:
        _CACHE["nc"] = build_bass()
    nc = _CACHE["nc"]
    in_maps = _prep_inputs(x, Wq, bq, Wk, bk, Wv, bv, Wo, bo)
    res = run_bass_kernel_spmd(nc, in_maps, list(range(W)), **run_kwargs)
    out = np.empty((TOK, D), np.float32)
    for c in range(W):
        r = res.results[c]["out"]
        for s in range(NSHIP):
            out[1024 * s + 128 * c:1024 * s + 128 * (c + 1)] = \
                r[128 * s:128 * (s + 1)]
    out = out.reshape(B, N, D) + bo.astype(np.float32)
    return out.astype(np.float32), res


def kernel(x, Wq, bq, Wk, bk, Wv, bv, Wo, bo):
    x, Wq, bq, Wk, bk, Wv, bv, Wo, bo = (
        np.asarray(a, dtype=np.float32)
        for a in (x, Wq, bq, Wk, bk, Wv, bv, Wo, bo)
    )
    out, _ = run(x, Wq, bq, Wk, bk, Wv, bv, Wo, bo)
    return out


# revision 19
# speedup vs baseline: 1.2240x; 1.0781x over previous
"""Multi-head attention forward on 8 Trainium2 NeuronCores (Bass/Tile).

Problem: B=2, N=2048, D=1024, H=16 heads of dh=64, fp32 in/out.

Sharding: tensor-parallel over heads - core c owns heads {2c, 2c+1} (one
128-row feature slice) for projections + attention over all 4096 tokens;
per-1024-token AllToAlls re-shard by token so each core runs the output
projection (full Wo) for its 512 tokens.

Speed recipe vs the fp32r version (316us):
 - fp16 operands everywhere with fp32 PSUM accumulation. fp32r matmuls
   measure ~432ns per 512-free matmul plus 287ns non-FWL LDWEIGHTS; fp16
   streams at full PE rate with fast-weight-load. End-to-end rounding error
   ~1e-3 against a 2e-2 budget.
 - scores: K=64 per head, so the two heads run as concurrent PE row-tiles
   (tile_position (0,0)/(64,0) via base partitions) instead of zero-padding
   the contraction to 128.
 - exp split: ScalarE does true exp for head0 + the tail of head1; VectorE
   computes the leading SCH_COLS of head1 with a Schraudolph bit-trick
   straight into fp16 bit patterns (i16 = round(1477.32*s + 15316), +-3%
   rel err, washes out under softmax normalization).
 - softmax denominators ride as a ones-column in the attn@v lhsT; head1
   uses [ones|v1] so its shifted SBUF copy lands dims on partitions 64..127
   with the denominator row just below them.
 - normalize multiplies run on GpSimd (SBUF-only engine, otherwise idle).
 - 4 AllToAlls of 256KB fp16 (one per 1024 tokens) pipelined behind the
   window loop; only the last is exposed in the tail.
 - single-DMA bulk loads: x / weights are host-permuted to [128, KC, *] so
   each chunk is one dma_start (DMA issue costs ~0.6us queue time each).
"""
from contextlib import ExitStack

import numpy as np

import concourse.bass as bass
import concourse.tile as tile
from concourse import bacc, mybir
from concourse.bass_utils import run_bass_kernel_spmd
from concourse.masks import make_identity

F32 = mybir.dt.float32
F16 = mybir.dt.float16
I16 = mybir.dt.int16

B, N, D, H, DH = 2, 2048, 1024, 16, 64
W = 8                    # cores
TOK = B * N              # 4096 flattened tokens
KC = D // 128            # contraction chunks for projections (8)
NW = TOK // 512          # 512-token n-windows (8)
MCB = N // 128           # m-chunks per batch (16)
NSHIP = 4                # one AllToAll per 1024 tokens

# Schraudolph fast-exp constants (fp16 bit space): i16 = round(A*s + B)
SCH_A = 1024 * 1.4426950408889634
SCH_B = 1024.0 * 15 - 44.0
SCH_COLS = 512           # leading cols of each e1 [128,512] tile on VectorE

_CACHE = {}


def build_bass():
    nc = bacc.Bacc("TRN2", target_bir_lowering=False)

    xT_d = nc.declare_dram_parameter("xT", [128, KC, TOK], F16, isOutput=False)
    wq_d = nc.declare_dram_parameter("wq", [128, KC, 128], F16, isOutput=False)
    wk_d = nc.declare_dram_parameter("wk", [128, KC, 128], F16, isOutput=False)
    wv_d = nc.declare_dram_parameter("wv", [128, KC, 128], F16, isOutput=False)
    wo_d = nc.declare_dram_parameter("wo", [128, KC, D], F16, isOutput=False)
    bqkv_d = nc.declare_dram_parameter("bqkv", [128, 3], F32, isOutput=False)
    out_d = nc.declare_dram_parameter("out", [512, D], F32, isOutput=True)

    a2a_in = [nc.dram_tensor(f"a2a_in{s}", [W, 128, 128], F16)
              for s in range(NSHIP)]
    a2a_out = [nc.dram_tensor(f"a2a_out{s}", [W, 128, 128], F16)
               for s in range(NSHIP)]
    a2a_wi = nc.dram_tensor("a2a_wi", [W, 1, 16], F16)
    a2a_wo = nc.dram_tensor("a2a_wo", [W, 1, 16], F16)

    with tile.TileContext(nc) as tc, ExitStack() as ctx:
        sb1 = ctx.enter_context(tc.tile_pool(name="sb1", bufs=1))
        sbe = ctx.enter_context(tc.tile_pool(name="sbe", bufs=2))
        sbx = ctx.enter_context(tc.tile_pool(name="sbx", bufs=4))
        ps_sc = ctx.enter_context(tc.tile_pool(name="ps_sc", bufs=3, space="PSUM"))
        ps_ha = ctx.enter_context(tc.tile_pool(name="ps_ha", bufs=1, space="PSUM"))
        ps_aux = ps_sc   # pj/tp/bc/op borrow the score rings ("sc0"/"sc1")

        # ---------- constants ----------
        ident = sb1.tile([128, 128], F16, tag="ident")
        make_identity(nc, ident[:])

        # selectors that broadcast softmax denominators across partitions
        sel0 = sb1.tile([65, 128], F16, tag="sel0")
        nc.vector.memset(sel0[:], 0.0)
        nc.vector.memset(sel0[64:65, 0:64], 1.0)
        sel1 = sb1.tile([128, 128], F16, tag="sel1")
        nc.vector.memset(sel1[:], 0.0)
        nc.vector.memset(sel1[32:33, 64:128], 1.0)

        bias = sb1.tile([128, 3], F32, tag="bias")
        nc.sync.dma_start(bias[:], bqkv_d[:])

        # ---------- persistent activations ----------
        qT = sb1.tile([128, TOK], F16, tag="qT")
        kT = sb1.tile([128, TOK], F16, tag="kT")     # rows 0-63 h0, 64-127 h1
        # v_aug[:, gm, 0:65]   = [v0 | ones] (denominator row at out 64)
        # v_aug[:, gm, 65:130] = [v1 | ones] (denominator row at out 64)
        v_aug = sb1.tile([128, 2 * MCB, 130], F16, tag="v_aug")
        nc.vector.memset(v_aug[:, :, 64:65], 1.0)
        nc.vector.memset(v_aug[:, :, 129:130], 1.0)
        heads = sb1.tile([128, TOK], F16, tag="heads")

        wq = sb1.tile([128, KC, 128], F16, tag="wq")
        wk = sb1.tile([128, KC, 128], F16, tag="wk")
        wv = sb1.tile([128, KC, 128], F16, tag="wv")
        wo = sb1.tile([128, KC, D], F16, tag="wo")
        hT = [sb1.tile([128, KC, 128], F16, tag=f"hT{s}", name=f"hT{s}")
              for s in range(NSHIP)]

        def load_x_chunk(t):
            xt = sbx.tile([128, KC, 512], F16, tag="xt", name=f"xt{t}")
            nc.sync.dma_start(xt[:], xT_d[:, :, bass.ts(t, 512)])
            return xt

        def proj_chunk(t, xt):
            """Project 512 tokens (global chunk t): q,k -> qT/kT, v -> v_aug."""
            tsl = bass.ts(t, 512)
            pq = ps_aux.tile([128, 512], F32, tag="sc0", name="pq")
            for k in range(KC):
                nc.tensor.matmul(pq[:], wq[:, k, :], xt[:, k, :],
                                 start=(k == 0), stop=(k == KC - 1))
            pk = ps_aux.tile([128, 512], F32, tag="sc1", name="pk")
            for k in range(KC):
                nc.tensor.matmul(pk[:], wk[:, k, :], xt[:, k, :],
                                 start=(k == 0), stop=(k == KC - 1))
            nc.vector.tensor_scalar_add(qT[:, tsl], pq[:], bias[:, 0:1])
            pv = ps_aux.tile([128, 512], F32, tag="sc0", name="pv")
            for k in range(KC):
                nc.tensor.matmul(pv[:], wv[:, k, :], xt[:, k, :],
                                 start=(k == 0), stop=(k == KC - 1))
            nc.scalar.add(kT[:, tsl], pk[:], bias[:, 1:2])
            vt = sbe.tile([128, 512], F16, tag="vt", name="vt")
            nc.vector.tensor_scalar_add(vt[:], pv[:], bias[:, 2:3])
            # transpose v 128-token-wise into v_aug rows (PE transpose)
            for i in range(4):
                gm = 4 * t + i
                tag = "sc0" if i % 2 == 0 else "sc1"
                tp = ps_aux.tile([128, 128], F16, tag=tag, name="tp")
                nc.tensor.transpose(tp[:], vt[:, bass.ts(i, 128)], ident[:])
                nc.scalar.copy(v_aug[:, gm, 0:64], tp[:, 0:64])
                nc.scalar.copy(v_aug[:, gm, 65:129], tp[:, 64:128])

        # ---------- stage 2 helpers ----------
        def emit_av(pr, last):
            e0, e1f, gm, ha0, ha1, _w = pr
            first = gm % MCB == 0
            # h1 first: its e comes from the (earlier-finishing) VectorE path
            nc.tensor.matmul(ha1[:], v_aug[:, gm, 65:130], e1f,
                             start=first, stop=last)
            nc.tensor.matmul(ha0[:], v_aug[:, gm, 0:65], e0[:],
                             start=first, stop=last)

        def emit_window_end(pr):
            """Copy the finished window's attn@v PSUM to SBUF (frees ha)."""
            _, _, _, ha0, ha1, w = pr
            hs0 = sbe.tile([65, 512], F16, tag="hs0", bufs=1)
            hs1 = sbe.tile([128, 512], F16, tag="hs1", bufs=1)
            nc.vector.tensor_copy(hs0[:], ha0[:])
            # v1 dims -> partitions 64..127; denominator row parked at 32
            nc.vector.tensor_copy(hs1[64:128, :], ha1[0:64, :])
            nc.vector.tensor_copy(hs1[32:33, :], ha1[64:65, :])
            return (hs0, hs1, w)

        def emit_normalize_bc(pend):
            hs0, hs1, w = pend
            bc = ps_aux.tile([128, 512], F32, tag="sc1", name="bc")
            nc.tensor.matmul(bc[:], sel0[:], hs0[:], start=True, stop=False)
            nc.tensor.matmul(bc[:], sel1[:], hs1[:], start=False, stop=True)
            bc_s = sbe.tile([128, 512], F32, tag="bc_s", bufs=1)
            nc.vector.reciprocal_approx_fast(bc_s[:], bc[:])
            return bc_s

        def emit_normalize_mul(pend, bc_s, tail=False):
            hs0, hs1, w = pend
            hsl = bass.ts(w, 512)
            eng0 = nc.vector if tail else nc.gpsimd
            eng0.tensor_mul(heads[0:64, hsl], hs0[0:64, :], bc_s[0:64, :])
            nc.gpsimd.tensor_mul(heads[64:128, hsl], hs1[64:128, :],
                                 bc_s[64:128, :])

        def emit_ship(s):
            for j in range(W):
                eng = nc.sync if j % 2 == 0 else nc.gpsimd
                eng.dma_start(a2a_in[s][j],
                              heads[:, bass.ds(1024 * s + 128 * j, 128)])
            nc.gpsimd.collective_compute(
                "AllToAll",
                mybir.AluOpType.bypass,
                ins=[a2a_in[s][:]],
                outs=[a2a_out[s][:]],
                replica_groups=[list(range(W))],
            )

        def emit_outproj(s):
            for j in range(W):
                eng = nc.sync if j % 2 == 0 else nc.gpsimd
                eng.dma_start(hT[s][:, j, :], a2a_out[s][j])
            for dc in range(2):
                op = ps_aux.tile([128, 512], F32, tag=("sc0", "sc1")[dc], name="op")
                for j in range(KC):
                    nc.tensor.matmul(op[:], hT[s][:, j, :],
                                     wo[:, j, bass.ts(dc, 512)],
                                     start=(j == 0), stop=(j == KC - 1))
                ot = sbe.tile([128, 512], F32, tag="ot")
                nc.scalar.copy(ot[:], op[:])
                nc.sync.dma_start(out_d[bass.ts(s, 128), bass.ts(dc, 512)], ot[:])

        def emit_exp(sc0, sc1, e0, e1i):
            nc.scalar.activation(e0[:], sc0[:], mybir.ActivationFunctionType.Exp)
            if SCH_COLS:
                nc.vector.tensor_scalar(
                    out=e1i[:, 0:SCH_COLS], in0=sc1[:, 0:SCH_COLS],
                    scalar1=SCH_A, scalar2=SCH_B,
                    op0=mybir.AluOpType.mult, op1=mybir.AluOpType.add)
            if SCH_COLS < 512:
                nc.scalar.activation(
                    e1i[:, SCH_COLS:512].bitcast(F16), sc1[:, SCH_COLS:512],
                    mybir.ActivationFunctionType.Exp)

        # ---------- schedule ----------
        # fire a tiny collective immediately: the first collective on the CC
        # stream pays ~40us of barrier/warmup cost - absorb it under stage 1
        nc.gpsimd.collective_compute(
            "AllToAll", mybir.AluOpType.bypass,
            ins=[a2a_wi[:]], outs=[a2a_wo[:]],
            replica_groups=[list(range(W))])
        nc.sync.dma_start(wq[:], wq_d[:])
        nc.sync.dma_start(wk[:], wk_d[:])
        nc.sync.dma_start(wv[:], wv_d[:])
        xts = {t: load_x_chunk(t) for t in range(2)}
        for t in range(4):          # batch-0 projections
            if t == 2:
                nc.sync.dma_start(wo[:], wo_d[:])
            if t + 2 < 4:
                xts[t + 2] = load_x_chunk(t + 2)
            proj_chunk(t, xts.pop(t))

        prev = None      # av software pipeline: (e0, e1f, gm, ha0, ha1, w)
        pending = None   # window awaiting normalize: (hs0, hs1, w)
        bc_pend = None   # (pend, bc_s)

        def window(w, pre=None):
            nonlocal prev, pending, bc_pend
            b = w // 4
            nsl = bass.ds(512 * w, 512)
            ha0 = ps_ha.tile([65, 512], F32, tag="ha0", name="ha0")
            ha1 = ps_ha.tile([65, 512], F32, tag="ha1", name="ha1")
            for mc in range(MCB):
                if mc == 0 and pre is not None:
                    pre()
                gm = MCB * b + mc
                msl = bass.ts(gm, 128)
                # score pair first (deps long-satisfied, so the scheduler
                # keeps them adjacent -> the two K=64 row-tiles run
                # concurrently), then the previous chunk's attn@v pair.
                sc0 = ps_sc.tile([128, 512], F32, tag="sc0", name="sc0")
                sc1 = ps_sc.tile([128, 512], F32, tag="sc1", name="sc1")
                nc.tensor.matmul(sc0[:], kT[0:64, msl], qT[0:64, nsl],
                                 start=True, stop=True)
                nc.tensor.matmul(sc1[:], kT[64:128, msl], qT[64:128, nsl],
                                 start=True, stop=True)
                if prev is not None:
                    last = prev[2] % MCB == MCB - 1
                    emit_av(prev, last)
                    if last:
                        pending = emit_window_end(prev)
                e0 = sbe.tile([128, 512], F16, tag="e0", name="e0")
                e1i = sbe.tile([128, 512], I16, tag="e1", name="e1")
                emit_exp(sc0, sc1, e0, e1i)
                prev = (e0, e1i[:].bitcast(F16), gm, ha0, ha1, w)
                if mc == 3 and pending is not None:
                    bc_pend = (pending, emit_normalize_bc(pending))
                    pending = None
                if mc == 5 and bc_pend is not None:
                    emit_normalize_mul(bc_pend[0], bc_pend[1])
                    pw = bc_pend[0][2]
                    bc_pend = None
                    if pw % 2 == 1:
                        emit_ship(pw // 2)
                # outproj(0) waits for the slowest (first real) collective -
                # stage it after the batch-1 projections instead of inside
                # window 3, so its PSUM-ring slot can't stall the scores.
                if mc == 2 and w == 4:
                    emit_outproj(0)
                if mc == 9 and w % 2 == 1 and w >= 5:
                    emit_outproj((w - 3) // 2)

        for w in range(4):          # batch-0 attention
            window(w, pre=(lambda t=w + 4: xts.__setitem__(t, load_x_chunk(t))))

        # batch-0 epilogue: finish window 3, normalize + ship it while the
        # batch-1 projections run
        emit_av(prev, True)
        pending = emit_window_end(prev)
        prev = None
        bc_s = emit_normalize_bc(pending)
        emit_normalize_mul(pending, bc_s)
        emit_ship(1)
        pending = None
        for t in range(4, 8):       # batch-1 projections
            proj_chunk(t, xts.pop(t))

        for w in range(4, 8):       # batch-1 attention
            window(w)

        # tail: window 7 normalize + final ship + out-projection
        emit_av(prev, True)
        pending = emit_window_end(prev)
        bc_s = emit_normalize_bc(pending)
        emit_normalize_mul(pending, bc_s, tail=True)
        emit_ship(3)
        # keep the PE busy (and its clock un-throttled) while the last
        # AllToAll flies; results are discarded
        for i in range(20):
            du = ps_aux.tile([128, 512], F32, tag=("sc0", "sc1")[i % 2], name="du")
            nc.tensor.matmul(du[:], kT[0:64, bass.ts(i % 8, 128)],
                             qT[0:64, 0:512], start=True, stop=True)
        emit_outproj(3)

    nc.compile()
    return nc


def _to_f16_perm(a):
    """[D, X] fp32 -> [128, KC, X] fp16 with rows regrouped per 128-block."""
    Dd, X = a.shape
    return np.ascontiguousarray(
        a.reshape(KC, 128, X).transpose(1, 0, 2)).astype(np.float16)


def _prep_inputs(x, Wq, bq, Wk, bk, Wv, bv, Wo, bo):
    xT = np.ascontiguousarray(x.reshape(TOK, D).T)
    xTr = _to_f16_perm(xT)
    wor = _to_f16_perm(Wo)
    in_maps = []
    for c in range(W):
        sl = slice(128 * c, 128 * (c + 1))
        bqkv = np.stack([bq[sl] / 8.0, bk[sl], bv[sl]], axis=1).astype(np.float32)
        in_maps.append({
            "xT": xTr,
            "wq": _to_f16_perm(np.ascontiguousarray(Wq[:, sl]) / 8.0),
            "wk": _to_f16_perm(np.ascontiguousarray(Wk[:, sl])),
            "wv": _to_f16_perm(np.ascontiguousarray(Wv[:, sl])),
            "wo": wor,
            "bqkv": np.ascontiguousarray(bqkv),
        })
    return in_maps


def run(x, Wq, bq, Wk, bk, Wv, bv, Wo, bo, **run_kwargs):
    if "nc" not in _CACHE:
        _CACHE["nc"] = build_bass()
    nc = _CACHE["nc"]
    in_maps = _prep_inputs(x, Wq, bq, Wk, bk, Wv, bv, Wo, bo)
    res = run_bass_kernel_spmd(nc, in_maps, list(range(W)), **run_kwargs)
    out = np.empty((TOK, D), np.float32)
    for c in range(W):
        r = res.results[c]["out"]
        for s in range(NSHIP):
            out[1024 * s + 128 * c:1024 * s + 128 * (c + 1)] = \
                r[128 * s:128 * (s + 1)]
    out = out.reshape(B, N, D) + bo.astype(np.float32)
    return out.astype(np.float32), res


def kernel(x, Wq, bq, Wk, bk, Wv, bv, Wo, bo):
    x, Wq, bq, Wk, bk, Wv, bv, Wo, bo = (
        np.asarray(a, dtype=np.float32)
        for a in (x, Wq, bq, Wk, bk, Wv, bv, Wo, bo)
    )
    out, _ = run(x, Wq, bq, Wk, bk, Wv, bv, Wo, bo)
    return out


# revision 23
# speedup vs baseline: 1.2310x; 1.0057x over previous
"""Multi-head attention forward on 8 Trainium2 NeuronCores (Bass/Tile).

Problem: B=2, N=2048, D=1024, H=16 heads of dh=64, fp32 in/out.

Sharding: tensor-parallel over heads - core c owns heads {2c, 2c+1} (one
128-row feature slice) for projections + attention over all 4096 tokens;
per-1024-token AllToAlls re-shard by token so each core runs the output
projection (full Wo) for its 512 tokens.

Speed recipe vs the fp32r version (316us):
 - fp16 operands everywhere with fp32 PSUM accumulation. fp32r matmuls
   measure ~432ns per 512-free matmul plus 287ns non-FWL LDWEIGHTS; fp16
   streams at full PE rate with fast-weight-load. End-to-end rounding error
   ~1e-3 against a 2e-2 budget.
 - scores: K=64 per head, so the two heads run as concurrent PE row-tiles
   (tile_position (0,0)/(64,0) via base partitions) instead of zero-padding
   the contraction to 128.
 - exp split: ScalarE does true exp for head0 + the tail of head1; VectorE
   computes the leading SCH_COLS of head1 with a Schraudolph bit-trick
   straight into fp16 bit patterns (i16 = round(1477.32*s + 15316), +-3%
   rel err, washes out under softmax normalization).
 - softmax denominators ride as a ones-column in the attn@v lhsT; head1
   uses [ones|v1] so its shifted SBUF copy lands dims on partitions 64..127
   with the denominator row just below them.
 - normalize multiplies run on GpSimd (SBUF-only engine, otherwise idle).
 - 4 AllToAlls of 256KB fp16 (one per 1024 tokens) pipelined behind the
   window loop; only the last is exposed in the tail.
 - single-DMA bulk loads: x / weights are host-permuted to [128, KC, *] so
   each chunk is one dma_start (DMA issue costs ~0.6us queue time each).
"""
from contextlib import ExitStack

import numpy as np

import concourse.bass as bass
import concourse.tile as tile
from concourse import bacc, mybir
from concourse.bass_utils import run_bass_kernel_spmd
from concourse.masks import make_identity

F32 = mybir.dt.float32
F16 = mybir.dt.float16
I16 = mybir.dt.int16

B, N, D, H, DH = 2, 2048, 1024, 16, 64
W = 8                    # cores
TOK = B * N              # 4096 flattened tokens
KC = D // 128            # contraction chunks for projections (8)
NW = TOK // 512          # 512-token n-windows (8)
MCB = N // 128           # m-chunks per batch (16)
NSHIP = 4                # one AllToAll per 1024 tokens

# Schraudolph fast-exp constants (fp16 bit space): i16 = round(A*s + B)
SCH_A = 1024 * 1.4426950408889634
SCH_B = 1024.0 * 15 - 44.0
SCH_COLS = 512           # leading cols of each e1 [128,512] tile on VectorE

_CACHE = {}


def build_bass():
    nc = bacc.Bacc("TRN2", target_bir_lowering=False)

    xT_d = nc.declare_dram_parameter("xT", [128, KC, TOK], F16, isOutput=False)
    wq_d = nc.declare_dram_parameter("wq", [128, KC, 128], F16, isOutput=False)
    wk_d = nc.declare_dram_parameter("wk", [128, KC, 128], F16, isOutput=False)
    wv_d = nc.declare_dram_parameter("wv", [128, KC, 128], F16, isOutput=False)
    wo_d = nc.declare_dram_parameter("wo", [128, KC, D], F16, isOutput=False)
    bqkv_d = nc.declare_dram_parameter("bqkv", [128, 3], F32, isOutput=False)
    out_d = nc.declare_dram_parameter("out", [512, D], F32, isOutput=True)

    a2a_in = [nc.dram_tensor(f"a2a_in{s}", [W, 128, 128], F16)
              for s in range(NSHIP)]
    a2a_out = [nc.dram_tensor(f"a2a_out{s}", [W, 128, 128], F16)
               for s in range(NSHIP)]
    a2a_wi = nc.dram_tensor("a2a_wi", [W, 1, 16], F16)
    a2a_wo = nc.dram_tensor("a2a_wo", [W, 1, 16], F16)

    with tile.TileContext(nc) as tc, ExitStack() as ctx:
        sb1 = ctx.enter_context(tc.tile_pool(name="sb1", bufs=1))
        sbe = ctx.enter_context(tc.tile_pool(name="sbe", bufs=2))
        sbx = ctx.enter_context(tc.tile_pool(name="sbx", bufs=4))
        ps_sc = ctx.enter_context(tc.tile_pool(name="ps_sc", bufs=3, space="PSUM"))
        ps_ha = ctx.enter_context(tc.tile_pool(name="ps_ha", bufs=1, space="PSUM"))
        ps_aux = ps_sc   # pj/tp/bc/op borrow the score rings ("sc0"/"sc1")

        # ---------- constants ----------
        ident = sb1.tile([128, 128], F16, tag="ident")
        make_identity(nc, ident[:])

        # selectors that broadcast softmax denominators across partitions
        sel0 = sb1.tile([65, 128], F16, tag="sel0")
        nc.vector.memset(sel0[:], 0.0)
        nc.vector.memset(sel0[64:65, 0:64], 1.0)
        sel1 = sb1.tile([128, 128], F16, tag="sel1")
        nc.vector.memset(sel1[:], 0.0)
        nc.vector.memset(sel1[32:33, 64:128], 1.0)

        bias = sb1.tile([128, 3], F32, tag="bias")
        nc.sync.dma_start(bias[:], bqkv_d[:])

        # ---------- persistent activations ----------
        qT = sb1.tile([128, TOK], F16, tag="qT")
        kT = sb1.tile([128, TOK], F16, tag="kT")     # rows 0-63 h0, 64-127 h1
        # v_aug[:, gm, 0:65]   = [v0 | ones] (denominator row at out 64)
        # v_aug[:, gm, 65:130] = [v1 | ones] (denominator row at out 64)
        v_aug = sb1.tile([128, 2 * MCB, 130], F16, tag="v_aug")
        nc.vector.memset(v_aug[:, :, 64:65], 1.0)
        nc.vector.memset(v_aug[:, :, 129:130], 1.0)
        heads = sb1.tile([128, TOK], F16, tag="heads")

        wq = sb1.tile([128, KC, 128], F16, tag="wq")
        wk = sb1.tile([128, KC, 128], F16, tag="wk")
        wv = sb1.tile([128, KC, 128], F16, tag="wv")
        wo = sb1.tile([128, KC, D], F16, tag="wo")
        hT = [sb1.tile([128, KC, 128], F16, tag=f"hT{s}", name=f"hT{s}")
              for s in range(NSHIP)]

        def load_x_chunk(t, split=False):
            xt = sbx.tile([128, KC, 512], F16, tag="xt", name=f"xt{t}")
            if split:
                # two DMAs so the first k-chunks land (and projections can
                # start) while the rest is still in flight
                nc.sync.dma_start(xt[:, 0:2, :], xT_d[:, 0:2, bass.ts(t, 512)])
                nc.sync.dma_start(xt[:, 2:KC, :], xT_d[:, 2:KC, bass.ts(t, 512)])
            else:
                nc.sync.dma_start(xt[:], xT_d[:, :, bass.ts(t, 512)])
            return xt

        def proj_chunk(t, xt):
            """Project 512 tokens (global chunk t): q,k -> qT/kT, v -> v_aug."""
            tsl = bass.ts(t, 512)
            pq = ps_aux.tile([128, 512], F32, tag="sc0", name="pq")
            for k in range(KC):
                nc.tensor.matmul(pq[:], wq[:, k, :], xt[:, k, :],
                                 start=(k == 0), stop=(k == KC - 1))
            pk = ps_aux.tile([128, 512], F32, tag="sc1", name="pk")
            for k in range(KC):
                nc.tensor.matmul(pk[:], wk[:, k, :], xt[:, k, :],
                                 start=(k == 0), stop=(k == KC - 1))
            nc.vector.tensor_scalar_add(qT[:, tsl], pq[:], bias[:, 0:1])
            pv = ps_aux.tile([128, 512], F32, tag="sc0", name="pv")
            for k in range(KC):
                nc.tensor.matmul(pv[:], wv[:, k, :], xt[:, k, :],
                                 start=(k == 0), stop=(k == KC - 1))
            nc.scalar.add(kT[:, tsl], pk[:], bias[:, 1:2])
            vt = sbe.tile([128, 512], F16, tag="vt", name="vt")
            nc.vector.tensor_scalar_add(vt[:], pv[:], bias[:, 2:3])
            # transpose v 128-token-wise into v_aug rows (PE transpose)
            for i in range(4):
                gm = 4 * t + i
                tag = "sc0" if i % 2 == 0 else "sc1"
                tp = ps_aux.tile([128, 128], F16, tag=tag, name="tp")
                nc.tensor.transpose(tp[:], vt[:, bass.ts(i, 128)], ident[:])
                nc.scalar.copy(v_aug[:, gm, 0:64], tp[:, 0:64])
                nc.scalar.copy(v_aug[:, gm, 65:129], tp[:, 64:128])

        # ---------- stage 2 helpers ----------
        def emit_av(pr, last):
            e0, e1f, gm, ha0, ha1, _w = pr
            first = gm % MCB == 0
            # h1 first: its e comes from the (earlier-finishing) VectorE path
            nc.tensor.matmul(ha1[:], v_aug[:, gm, 65:130], e1f,
                             start=first, stop=last)
            nc.tensor.matmul(ha0[:], v_aug[:, gm, 0:65], e0[:],
                             start=first, stop=last)

        def emit_window_end(pr):
            """Copy the finished window's attn@v PSUM to SBUF (frees ha)."""
            _, _, _, ha0, ha1, w = pr
            hs0 = sbe.tile([65, 512], F16, tag="hs0", bufs=1)
            hs1 = sbe.tile([128, 512], F16, tag="hs1", bufs=1)
            nc.vector.tensor_copy(hs0[:], ha0[:])
            # v1 dims -> partitions 64..127; denominator row parked at 32
            nc.vector.tensor_copy(hs1[64:128, :], ha1[0:64, :])
            nc.vector.tensor_copy(hs1[32:33, :], ha1[64:65, :])
            return (hs0, hs1, w)

        def emit_normalize_bc(pend):
            hs0, hs1, w = pend
            bc = ps_aux.tile([128, 512], F32, tag="sc1", name="bc")
            nc.tensor.matmul(bc[:], sel0[:], hs0[:], start=True, stop=False)
            nc.tensor.matmul(bc[:], sel1[:], hs1[:], start=False, stop=True)
            bc_s = sbe.tile([128, 512], F32, tag="bc_s", bufs=1)
            nc.vector.reciprocal_approx_fast(bc_s[:], bc[:])
            return bc_s

        def emit_normalize_mul(pend, bc_s, tail=False):
            hs0, hs1, w = pend
            hsl = bass.ts(w, 512)
            eng0 = nc.vector if tail else nc.gpsimd
            eng0.tensor_mul(heads[0:64, hsl], hs0[0:64, :], bc_s[0:64, :])
            nc.gpsimd.tensor_mul(heads[64:128, hsl], hs1[64:128, :],
                                 bc_s[64:128, :])

        def emit_ship(s):
            for j in range(W):
                eng = nc.sync if j % 2 == 0 else nc.gpsimd
                eng.dma_start(a2a_in[s][j],
                              heads[:, bass.ds(1024 * s + 128 * j, 128)])
            nc.gpsimd.collective_compute(
                "AllToAll",
                mybir.AluOpType.bypass,
                ins=[a2a_in[s][:]],
                outs=[a2a_out[s][:]],
                replica_groups=[list(range(W))],
            )

        def emit_outproj(s, pool=None, tags=("sc0", "sc1")):
            pool = pool or ps_aux
            for j in range(W):
                eng = nc.sync if j % 2 == 0 else nc.gpsimd
                eng.dma_start(hT[s][:, j, :], a2a_out[s][j])
            ops = [pool.tile([128, 512], F32, tag=tags[dc], name="op")
                   for dc in range(2)]
            # interleave the two accumulation chains so consecutive matmuls
            # never share a PSUM bank and pipeline their drains
            for j in range(KC):
                for dc in range(2):
                    nc.tensor.matmul(ops[dc][:], hT[s][:, j, :],
                                     wo[:, j, bass.ts(dc, 512)],
                                     start=(j == 0), stop=(j == KC - 1))
            for dc in range(2):
                ot = sbe.tile([128, 512], F32, tag="ot")
                nc.scalar.copy(ot[:], ops[dc][:])
                nc.sync.dma_start(out_d[bass.ts(s, 128), bass.ts(dc, 512)], ot[:])

        def emit_exp(sc0, sc1, e0, e1i):
            nc.scalar.activation(e0[:], sc0[:], mybir.ActivationFunctionType.Exp)
            if SCH_COLS:
                nc.vector.tensor_scalar(
                    out=e1i[:, 0:SCH_COLS], in0=sc1[:, 0:SCH_COLS],
                    scalar1=SCH_A, scalar2=SCH_B,
                    op0=mybir.AluOpType.mult, op1=mybir.AluOpType.add)
            if SCH_COLS < 512:
                nc.scalar.activation(
                    e1i[:, SCH_COLS:512].bitcast(F16), sc1[:, SCH_COLS:512],
                    mybir.ActivationFunctionType.Exp)

        # ---------- schedule ----------
        # fire a tiny collective immediately: the first collective on the CC
        # stream pays ~40us of barrier/warmup cost - absorb it under stage 1
        nc.gpsimd.collective_compute(
            "AllToAll", mybir.AluOpType.bypass,
            ins=[a2a_wi[:]], outs=[a2a_wo[:]],
            replica_groups=[list(range(W))])
        nc.sync.dma_start(wq[:], wq_d[:])
        xts = {0: load_x_chunk(0, split=True)}
        nc.sync.dma_start(wk[:], wk_d[:])
        nc.sync.dma_start(wv[:], wv_d[:])
        xts[1] = load_x_chunk(1)
        for t in range(4):          # batch-0 projections
            if t == 2:
                nc.sync.dma_start(wo[:], wo_d[:])
            if t + 2 < 4:
                xts[t + 2] = load_x_chunk(t + 2)
            proj_chunk(t, xts.pop(t))

        prev = None      # av software pipeline: (e0, e1f, gm, ha0, ha1, w)
        pending = None   # window awaiting normalize: (hs0, hs1, w)
        bc_pend = None   # (pend, bc_s)

        def window(w, pre=None):
            nonlocal prev, pending, bc_pend
            b = w // 4
            nsl = bass.ds(512 * w, 512)
            ha0 = ps_ha.tile([65, 512], F32, tag="ha0", name="ha0")
            ha1 = ps_ha.tile([65, 512], F32, tag="ha1", name="ha1")
            for mc in range(MCB):
                if mc == 0 and pre is not None:
                    pre()
                gm = MCB * b + mc
                msl = bass.ts(gm, 128)
                # score pair first (deps long-satisfied, so the scheduler
                # keeps them adjacent -> the two K=64 row-tiles run
                # concurrently), then the previous chunk's attn@v pair.
                sc0 = ps_sc.tile([128, 512], F32, tag="sc0", name="sc0")
                sc1 = ps_sc.tile([128, 512], F32, tag="sc1", name="sc1")
                nc.tensor.matmul(sc0[:], kT[0:64, msl], qT[0:64, nsl],
                                 start=True, stop=True)
                nc.tensor.matmul(sc1[:], kT[64:128, msl], qT[64:128, nsl],
                                 start=True, stop=True)
                if prev is not None:
                    last = prev[2] % MCB == MCB - 1
                    emit_av(prev, last)
                    if last:
                        pending = emit_window_end(prev)
                e0 = sbe.tile([128, 512], F16, tag="e0", name="e0")
                e1i = sbe.tile([128, 512], I16, tag="e1", name="e1")
                emit_exp(sc0, sc1, e0, e1i)
                prev = (e0, e1i[:].bitcast(F16), gm, ha0, ha1, w)
                if mc == 3 and pending is not None:
                    bc_pend = (pending, emit_normalize_bc(pending))
                    pending = None
                if mc == 5 and bc_pend is not None:
                    emit_normalize_mul(bc_pend[0], bc_pend[1])
                    pw = bc_pend[0][2]
                    bc_pend = None
                    if pw % 2 == 1:
                        emit_ship(pw // 2)
                # outproj(0) waits for the slowest (first real) collective -
                # stage it after the batch-1 projections instead of inside
                # window 3, so its PSUM-ring slot can't stall the scores.
                if mc == 2 and w == 4:
                    emit_outproj(0)
                if mc == 9 and w % 2 == 1 and w >= 5:
                    emit_outproj((w - 3) // 2)

        for w in range(4):          # batch-0 attention
            window(w, pre=(lambda t=w + 4: xts.__setitem__(t, load_x_chunk(t))))

        # batch-0 epilogue: finish window 3, normalize + ship it while the
        # batch-1 projections run
        emit_av(prev, True)
        pending = emit_window_end(prev)
        prev = None
        bc_s = emit_normalize_bc(pending)
        emit_normalize_mul(pending, bc_s)
        emit_ship(1)
        pending = None
        for t in range(4, 8):       # batch-1 projections
            proj_chunk(t, xts.pop(t))

        for w in range(4, 8):       # batch-1 attention
            window(w)

        # tail: window 7 normalize + final ship + out-projection
        emit_av(prev, True)
        pending = emit_window_end(prev)
        bc_s = emit_normalize_bc(pending)
        emit_normalize_mul(pending, bc_s, tail=True)
        emit_ship(3)
        # keep the PE busy (and its clock un-throttled) while the last
        # AllToAll flies; results are discarded. Reading the tail of `heads`
        # gates these behind the final normalize so the scheduler cannot
        # hoist them into the window loop.
        for i in range(24):
            du = ps_ha.tile([128, 512], F32, tag=("ha0", "ha1")[i % 2], name="du")
            nc.tensor.matmul(du[:], kT[0:64, bass.ts(i % 8, 128)],
                             heads[0:64, bass.ds(3584, 512)],
                             start=True, stop=True)
        emit_outproj(3, pool=ps_ha, tags=("ha0", "ha1"))

    nc.compile()
    return nc


def _to_f16_perm(a):
    """[D, X] fp32 -> [128, KC, X] fp16 with rows regrouped per 128-block."""
    Dd, X = a.shape
    return np.ascontiguousarray(
        a.reshape(KC, 128, X).transpose(1, 0, 2)).astype(np.float16)


def _prep_inputs(x, Wq, bq, Wk, bk, Wv, bv, Wo, bo):
    xT = np.ascontiguousarray(x.reshape(TOK, D).T)
    xTr = _to_f16_perm(xT)
    wor = _to_f16_perm(Wo)
    in_maps = []
    for c in range(W):
        sl = slice(128 * c, 128 * (c + 1))
        bqkv = np.stack([bq[sl] / 8.0, bk[sl], bv[sl]], axis=1).astype(np.float32)
        in_maps.append({
            "xT": xTr,
            "wq": _to_f16_perm(np.ascontiguousarray(Wq[:, sl]) / 8.0),
            "wk": _to_f16_perm(np.ascontiguousarray(Wk[:, sl])),
            "wv": _to_f16_perm(np.ascontiguousarray(Wv[:, sl])),
            "wo": wor,
            "bqkv": np.ascontiguousarray(bqkv),
        })
    return in_maps


def run(x, Wq, bq, Wk, bk, Wv, bv, Wo, bo, **run_kwargs):
    if "nc" not in _CACHE:
        _CACHE["nc"] = build_bass()
    nc = _CACHE["nc"]
    in_maps = _prep_inputs(x, Wq, bq, Wk, bk, Wv, bv, Wo, bo)
    res = run_bass_kernel_spmd(nc, in_maps, list(range(W)), **run_kwargs)
    out = np.empty((TOK, D), np.float32)
    for c in range(W):
        r = res.results[c]["out"]
        for s in range(NSHIP):
            out[1024 * s + 128 * c:1024 * s + 128 * (c + 1)] = \
                r[128 * s:128 * (s + 1)]
    out = out.reshape(B, N, D) + bo.astype(np.float32)
    return out.astype(np.float32), res


def kernel(x, Wq, bq, Wk, bk, Wv, bv, Wo, bo):
    x, Wq, bq, Wk, bk, Wv, bv, Wo, bo = (
        np.asarray(a, dtype=np.float32)
        for a in (x, Wq, bq, Wk, bk, Wv, bv, Wo, bo)
    )
    out, _ = run(x, Wq, bq, Wk, bk, Wv, bv, Wo, bo)
    return out


# revision 25
# speedup vs baseline: 1.3368x; 1.0859x over previous
"""Multi-head attention forward on 8 Trainium2 NeuronCores (Bass/Tile).

Problem: B=2, N=2048, D=1024, H=16 heads of dh=64, fp32 in/out.

Sharding: tensor-parallel over heads - core c owns heads {2c, 2c+1} (one
128-row feature slice) for projections + attention over all 4096 tokens;
per-1024-token AllToAlls re-shard by token so each core runs the output
projection (full Wo) for its 512 tokens.

Speed recipe vs the fp32r version (316us):
 - fp16 operands everywhere with fp32 PSUM accumulation. fp32r matmuls
   measure ~432ns per 512-free matmul plus 287ns non-FWL LDWEIGHTS; fp16
   streams at full PE rate with fast-weight-load. End-to-end rounding error
   ~1e-3 against a 2e-2 budget.
 - scores: K=64 per head, so the two heads run as concurrent PE row-tiles
   (tile_position (0,0)/(64,0) via base partitions) instead of zero-padding
   the contraction to 128.
 - exp split: ScalarE does true exp for head0 + the tail of head1; VectorE
   computes the leading SCH_COLS of head1 with a Schraudolph bit-trick
   straight into fp16 bit patterns (i16 = round(1477.32*s + 15316), +-3%
   rel err, washes out under softmax normalization).
 - softmax denominators ride as a ones-column in the attn@v lhsT; head1
   uses [ones|v1] so its shifted SBUF copy lands dims on partitions 64..127
   with the denominator row just below them.
 - normalize multiplies run on GpSimd (SBUF-only engine, otherwise idle).
 - 4 AllToAlls of 256KB fp16 (one per 1024 tokens) pipelined behind the
   window loop; only the last is exposed in the tail.
 - single-DMA bulk loads: x / weights are host-permuted to [128, KC, *] so
   each chunk is one dma_start (DMA issue costs ~0.6us queue time each).
"""
from contextlib import ExitStack

import numpy as np

import concourse.bass as bass
import concourse.tile as tile
from concourse import bacc, mybir
from concourse.bass_utils import run_bass_kernel_spmd
from concourse.masks import make_identity

F32 = mybir.dt.float32
F16 = mybir.dt.float16
I16 = mybir.dt.int16

B, N, D, H, DH = 2, 2048, 1024, 16, 64
W = 8                    # cores
TOK = B * N              # 4096 flattened tokens
KC = D // 128            # contraction chunks for projections (8)
NW = TOK // 512          # 512-token n-windows (8)
MCB = N // 128           # m-chunks per batch (16)
NSHIP = 4                # one AllToAll per 1024 tokens

# Schraudolph fast-exp constants (fp16 bit space): i16 = round(A*s + B)
SCH_A = 1024 * 1.4426950408889634
SCH_B = 1024.0 * 15 - 44.0
SCH_COLS = 512           # leading cols of each e1 [128,512] tile on VectorE

_CACHE = {}


def build_bass():
    nc = bacc.Bacc("TRN2", target_bir_lowering=False)

    xT_d = nc.declare_dram_parameter("xT", [128, KC, TOK], F16, isOutput=False)
    wq_d = nc.declare_dram_parameter("wq", [128, KC, 128], F16, isOutput=False)
    wk_d = nc.declare_dram_parameter("wk", [128, KC, 128], F16, isOutput=False)
    wv_d = nc.declare_dram_parameter("wv", [128, KC, 128], F16, isOutput=False)
    wo_d = nc.declare_dram_parameter("wo", [128, KC, D], F16, isOutput=False)
    bqkv_d = nc.declare_dram_parameter("bqkv", [128, 3], F32, isOutput=False)
    out_d = nc.declare_dram_parameter("out", [512, D], F32, isOutput=True)

    a2a_in = [nc.dram_tensor(f"a2a_in{s}", [W, 128, 128], F16)
              for s in range(NSHIP)]
    a2a_out = [nc.dram_tensor(f"a2a_out{s}", [W, 128, 128], F16)
               for s in range(NSHIP)]
    a2a_wi = nc.dram_tensor("a2a_wi", [W, 1, 16], F16)
    a2a_wo = nc.dram_tensor("a2a_wo", [W, 1, 16], F16)

    with tile.TileContext(nc) as tc, ExitStack() as ctx:
        sb1 = ctx.enter_context(tc.tile_pool(name="sb1", bufs=1))
        sbe = ctx.enter_context(tc.tile_pool(name="sbe", bufs=2))
        sbx = ctx.enter_context(tc.tile_pool(name="sbx", bufs=4))
        ps_sc = ctx.enter_context(tc.tile_pool(name="ps_sc", bufs=3, space="PSUM"))
        ps_ha = ctx.enter_context(tc.tile_pool(name="ps_ha", bufs=1, space="PSUM"))
        ps_aux = ps_sc   # pj/tp/bc/op borrow the score rings ("sc0"/"sc1")

        # ---------- constants ----------
        ident = sb1.tile([128, 128], F16, tag="ident")
        make_identity(nc, ident[:])

        # selectors that broadcast softmax denominators across partitions
        sel0 = sb1.tile([65, 128], F16, tag="sel0")
        nc.vector.memset(sel0[:], 0.0)
        nc.vector.memset(sel0[64:65, 0:64], 1.0)
        sel1 = sb1.tile([128, 128], F16, tag="sel1")
        nc.vector.memset(sel1[:], 0.0)
        nc.vector.memset(sel1[32:33, 64:128], 1.0)

        bias = sb1.tile([128, 3], F32, tag="bias")
        nc.sync.dma_start(bias[:], bqkv_d[:])

        # ---------- persistent activations ----------
        qT = sb1.tile([128, TOK], F16, tag="qT")
        kT = sb1.tile([128, TOK], F16, tag="kT")     # rows 0-63 h0, 64-127 h1
        # v_aug[:, gm, 0:65]   = [v0 | ones] (denominator row at out 64)
        # v_aug[:, gm, 65:130] = [v1 | ones] (denominator row at out 64)
        v_aug = sb1.tile([128, 2 * MCB, 130], F16, tag="v_aug")
        nc.vector.memset(v_aug[:, :, 64:65], 1.0)
        nc.vector.memset(v_aug[:, :, 129:130], 1.0)
        heads = sb1.tile([128, TOK], F16, tag="heads")

        wq = sb1.tile([128, KC, 128], F16, tag="wq")
        wk = sb1.tile([128, KC, 128], F16, tag="wk")
        wv = sb1.tile([128, KC, 128], F16, tag="wv")
        wo = sb1.tile([128, KC, D], F16, tag="wo")
        hT = [sb1.tile([128, KC, 128], F16, tag=f"hT{s}", name=f"hT{s}")
              for s in range(NSHIP)]

        def load_x_chunk(t, split=False):
            xt = sbx.tile([128, KC, 512], F16, tag="xt", name=f"xt{t}")
            if split:
                # two DMAs so the first k-chunks land (and projections can
                # start) while the rest is still in flight
                nc.sync.dma_start(xt[:, 0:2, :], xT_d[:, 0:2, bass.ts(t, 512)])
                nc.sync.dma_start(xt[:, 2:KC, :], xT_d[:, 2:KC, bass.ts(t, 512)])
            else:
                nc.sync.dma_start(xt[:], xT_d[:, :, bass.ts(t, 512)])
            return xt

        def proj_chunk(t, xt):
            """Project 512 tokens (global chunk t): q,k -> qT/kT, v -> v_aug."""
            tsl = bass.ts(t, 512)
            pq = ps_aux.tile([128, 512], F32, tag="sc0", name="pq")
            for k in range(KC):
                nc.tensor.matmul(pq[:], wq[:, k, :], xt[:, k, :],
                                 start=(k == 0), stop=(k == KC - 1))
            pk = ps_aux.tile([128, 512], F32, tag="sc1", name="pk")
            for k in range(KC):
                nc.tensor.matmul(pk[:], wk[:, k, :], xt[:, k, :],
                                 start=(k == 0), stop=(k == KC - 1))
            nc.vector.tensor_scalar_add(qT[:, tsl], pq[:], bias[:, 0:1])
            pv = ps_aux.tile([128, 512], F32, tag="sc0", name="pv")
            for k in range(KC):
                nc.tensor.matmul(pv[:], wv[:, k, :], xt[:, k, :],
                                 start=(k == 0), stop=(k == KC - 1))
            nc.scalar.add(kT[:, tsl], pk[:], bias[:, 1:2])
            vt = sbe.tile([128, 512], F16, tag="vt", name="vt")
            nc.vector.tensor_scalar_add(vt[:], pv[:], bias[:, 2:3])
            # transpose v 128-token-wise into v_aug rows (PE transpose)
            for i in range(4):
                gm = 4 * t + i
                tag = "sc0" if i % 2 == 0 else "sc1"
                tp = ps_aux.tile([128, 128], F16, tag=tag, name="tp")
                nc.tensor.transpose(tp[:], vt[:, bass.ts(i, 128)], ident[:])
                nc.scalar.copy(v_aug[:, gm, 0:64], tp[:, 0:64])
                nc.scalar.copy(v_aug[:, gm, 65:129], tp[:, 64:128])

        # ---------- stage 2 helpers ----------
        def emit_av(pr, last):
            e0, e1f, gm, ha0, ha1, _w = pr
            first = gm % MCB == 0
            # h1 first: its e comes from the (earlier-finishing) VectorE path
            nc.tensor.matmul(ha1[:], v_aug[:, gm, 65:130], e1f,
                             start=first, stop=last)
            nc.tensor.matmul(ha0[:], v_aug[:, gm, 0:65], e0[:],
                             start=first, stop=last)

        def emit_window_end(pr):
            """Copy the finished window's attn@v PSUM to SBUF (frees ha)."""
            _, _, _, ha0, ha1, w = pr
            hs0 = sbe.tile([65, 512], F16, tag="hs0", bufs=1)
            hs1 = sbe.tile([128, 512], F16, tag="hs1", bufs=1)
            nc.vector.tensor_copy(hs0[:], ha0[:])
            # v1 dims -> partitions 64..127; denominator row parked at 32
            nc.vector.tensor_copy(hs1[64:128, :], ha1[0:64, :])
            nc.vector.tensor_copy(hs1[32:33, :], ha1[64:65, :])
            return (hs0, hs1, w)

        def emit_normalize_bc(pend):
            hs0, hs1, w = pend
            bc = ps_aux.tile([128, 512], F32, tag="sc1", name="bc")
            nc.tensor.matmul(bc[:], sel0[:], hs0[:], start=True, stop=False)
            nc.tensor.matmul(bc[:], sel1[:], hs1[:], start=False, stop=True)
            bc_s = sbe.tile([128, 512], F32, tag="bc_s", bufs=1)
            nc.vector.reciprocal_approx_fast(bc_s[:], bc[:])
            return bc_s

        def emit_normalize_mul(pend, bc_s, tail=False):
            hs0, hs1, w = pend
            hsl = bass.ts(w, 512)
            eng0 = nc.vector if tail else nc.gpsimd
            eng0.tensor_mul(heads[0:64, hsl], hs0[0:64, :], bc_s[0:64, :])
            nc.gpsimd.tensor_mul(heads[64:128, hsl], hs1[64:128, :],
                                 bc_s[64:128, :])

        def emit_ship(s):
            for j in range(W):
                eng = nc.sync if j % 2 == 0 else nc.gpsimd
                eng.dma_start(a2a_in[s][j],
                              heads[:, bass.ds(1024 * s + 128 * j, 128)])
            nc.gpsimd.collective_compute(
                "AllToAll",
                mybir.AluOpType.bypass,
                ins=[a2a_in[s][:]],
                outs=[a2a_out[s][:]],
                replica_groups=[list(range(W))],
            )

        def emit_outproj(s, pool=None, tags=("sc0", "sc1")):
            pool = pool or ps_aux
            for j in range(W):
                eng = nc.sync if j % 2 == 0 else nc.gpsimd
                eng.dma_start(hT[s][:, j, :], a2a_out[s][j])
            ops = [pool.tile([128, 512], F32, tag=tags[dc], name="op")
                   for dc in range(2)]
            # interleave the two accumulation chains so consecutive matmuls
            # never share a PSUM bank and pipeline their drains
            for j in range(KC):
                for dc in range(2):
                    nc.tensor.matmul(ops[dc][:], hT[s][:, j, :],
                                     wo[:, j, bass.ts(dc, 512)],
                                     start=(j == 0), stop=(j == KC - 1))
            for dc in range(2):
                ot = sbe.tile([128, 512], F32, tag="ot")
                nc.scalar.copy(ot[:], ops[dc][:])
                nc.sync.dma_start(out_d[bass.ts(s, 128), bass.ts(dc, 512)], ot[:])

        def emit_exp(sc0, sc1, e0, e1i):
            nc.scalar.activation(e0[:], sc0[:], mybir.ActivationFunctionType.Exp)
            if SCH_COLS:
                nc.vector.tensor_scalar(
                    out=e1i[:, 0:SCH_COLS], in0=sc1[:, 0:SCH_COLS],
                    scalar1=SCH_A, scalar2=SCH_B,
                    op0=mybir.AluOpType.mult, op1=mybir.AluOpType.add)
            if SCH_COLS < 512:
                nc.scalar.activation(
                    e1i[:, SCH_COLS:512].bitcast(F16), sc1[:, SCH_COLS:512],
                    mybir.ActivationFunctionType.Exp)

        # ---------- schedule ----------
        # fire a tiny collective immediately: the first collective on the CC
        # stream pays ~40us of barrier/warmup cost - absorb it under stage 1
        nc.gpsimd.collective_compute(
            "AllToAll", mybir.AluOpType.bypass,
            ins=[a2a_wi[:]], outs=[a2a_wo[:]],
            replica_groups=[list(range(W))])
        nc.sync.dma_start(wq[:], wq_d[:])
        xts = {0: load_x_chunk(0, split=True)}
        nc.sync.dma_start(wk[:], wk_d[:])
        nc.sync.dma_start(wv[:], wv_d[:])
        xts[1] = load_x_chunk(1)
        for t in range(4):          # batch-0 projections
            if t == 2:
                nc.sync.dma_start(wo[:], wo_d[:])
            if t + 2 < 4:
                xts[t + 2] = load_x_chunk(t + 2)
            proj_chunk(t, xts.pop(t))

        prev = None      # av software pipeline: (e0, e1f, gm, ha0, ha1, w)
        pending = None   # window awaiting normalize: (hs0, hs1, w)
        bc_pend = None   # (pend, bc_s)

        def window(w, pre=None):
            nonlocal prev, pending, bc_pend
            b = w // 4
            nsl = bass.ds(512 * w, 512)
            ha0 = ps_ha.tile([65, 512], F32, tag="ha0", name="ha0")
            ha1 = ps_ha.tile([65, 512], F32, tag="ha1", name="ha1")
            for mc in range(MCB):
                if mc == 0 and pre is not None:
                    pre()
                gm = MCB * b + mc
                msl = bass.ts(gm, 128)
                # score pair first (deps long-satisfied, so the scheduler
                # keeps them adjacent -> the two K=64 row-tiles run
                # concurrently), then the previous chunk's attn@v pair.
                sc0 = ps_sc.tile([128, 512], F32, tag="sc0", name="sc0")
                sc1 = ps_sc.tile([128, 512], F32, tag="sc1", name="sc1")
                nc.tensor.matmul(sc0[:], kT[0:64, msl], qT[0:64, nsl],
                                 start=True, stop=True)
                nc.tensor.matmul(sc1[:], kT[64:128, msl], qT[64:128, nsl],
                                 start=True, stop=True)
                if prev is not None:
                    last = prev[2] % MCB == MCB - 1
                    emit_av(prev, last)
                    if last:
                        pending = emit_window_end(prev)
                e0 = sbe.tile([128, 512], F16, tag="e0", name="e0")
                e1i = sbe.tile([128, 512], I16, tag="e1", name="e1")
                emit_exp(sc0, sc1, e0, e1i)
                prev = (e0, e1i[:].bitcast(F16), gm, ha0, ha1, w)
                if mc == 3 and pending is not None:
                    bc_pend = (pending, emit_normalize_bc(pending))
                    pending = None
                if mc == 5 and bc_pend is not None:
                    emit_normalize_mul(bc_pend[0], bc_pend[1])
                    pw = bc_pend[0][2]
                    bc_pend = None
                    if pw % 2 == 1:
                        emit_ship(pw // 2)
                # outproj(0) waits for the slowest (first real) collective -
                # stage it after the batch-1 projections instead of inside
                # window 3, so its PSUM-ring slot can't stall the scores.
                # outproj(2) moves to the tail: emitted inside window 7 its
                # collective-blocked hT DMAs head-of-line block the gpsimd
                # queue and delay the final ship by ~30us.
                if mc == 2 and w == 4:
                    emit_outproj(0)
                if mc == 9 and w == 5:
                    emit_outproj(1)

        for w in range(4):          # batch-0 attention
            window(w, pre=(lambda t=w + 4: xts.__setitem__(t, load_x_chunk(t))))

        # batch-0 epilogue: finish window 3, normalize + ship it while the
        # batch-1 projections run
        emit_av(prev, True)
        pending = emit_window_end(prev)
        prev = None
        bc_s = emit_normalize_bc(pending)
        emit_normalize_mul(pending, bc_s)
        emit_ship(1)
        pending = None
        for t in range(4, 8):       # batch-1 projections
            proj_chunk(t, xts.pop(t))

        for w in range(4, 8):       # batch-1 attention
            window(w)

        # tail: window 7 normalize + final ship + out-projection
        emit_av(prev, True)
        pending = emit_window_end(prev)
        bc_s = emit_normalize_bc(pending)
        emit_normalize_mul(pending, bc_s, tail=True)
        emit_ship(3)
        # now that the final collective is in flight: out-projections for
        # shipments 2 and 3, with clock-warming dummies (gated on the tail
        # of `heads` so the scheduler cannot hoist them) filling the
        # collective-wait gaps; results of the dummies are discarded.
        emit_outproj(2)
        for i in range(24):
            du = ps_ha.tile([128, 512], F32, tag=("ha0", "ha1")[i % 2], name="du")
            nc.tensor.matmul(du[:], kT[0:64, bass.ts(i % 8, 128)],
                             heads[0:64, bass.ds(3584, 512)],
                             start=True, stop=True)
        emit_outproj(3, pool=ps_ha, tags=("ha0", "ha1"))

    nc.compile()
    return nc


def _to_f16_perm(a):
    """[D, X] fp32 -> [128, KC, X] fp16 with rows regrouped per 128-block."""
    Dd, X = a.shape
    return np.ascontiguousarray(
        a.reshape(KC, 128, X).transpose(1, 0, 2)).astype(np.float16)


def _prep_inputs(x, Wq, bq, Wk, bk, Wv, bv, Wo, bo):
    xT = np.ascontiguousarray(x.reshape(TOK, D).T)
    xTr = _to_f16_perm(xT)
    wor = _to_f16_perm(Wo)
    in_maps = []
    for c in range(W):
        sl = slice(128 * c, 128 * (c + 1))
        bqkv = np.stack([bq[sl] / 8.0, bk[sl], bv[sl]], axis=1).astype(np.float32)
        in_maps.append({
            "xT": xTr,
            "wq": _to_f16_perm(np.ascontiguousarray(Wq[:, sl]) / 8.0),
            "wk": _to_f16_perm(np.ascontiguousarray(Wk[:, sl])),
            "wv": _to_f16_perm(np.ascontiguousarray(Wv[:, sl])),
            "wo": wor,
            "bqkv": np.ascontiguousarray(bqkv),
        })
    return in_maps


def run(x, Wq, bq, Wk, bk, Wv, bv, Wo, bo, **run_kwargs):
    if "nc" not in _CACHE:
        _CACHE["nc"] = build_bass()
    nc = _CACHE["nc"]
    in_maps = _prep_inputs(x, Wq, bq, Wk, bk, Wv, bv, Wo, bo)
    res = run_bass_kernel_spmd(nc, in_maps, list(range(W)), **run_kwargs)
    out = np.empty((TOK, D), np.float32)
    for c in range(W):
        r = res.results[c]["out"]
        for s in range(NSHIP):
            out[1024 * s + 128 * c:1024 * s + 128 * (c + 1)] = \
                r[128 * s:128 * (s + 1)]
    out = out.reshape(B, N, D) + bo.astype(np.float32)
    return out.astype(np.float32), res


def kernel(x, Wq, bq, Wk, bk, Wv, bv, Wo, bo):
    x, Wq, bq, Wk, bk, Wv, bv, Wo, bo = (
        np.asarray(a, dtype=np.float32)
        for a in (x, Wq, bq, Wk, bk, Wv, bv, Wo, bo)
    )
    out, _ = run(x, Wq, bq, Wk, bk, Wv, bv, Wo, bo)
    return out
